# revision 1
# baseline (speedup 1.0000x reference)
"""Trainium2 Bass kernel for nn_BondMatrixMessage (GNN bond-matrix message passing).

Per batch b (one NeuronCore each, B=8 => 8 cores):
    bw[e,(i,j)] = sum_k bond[e,k] * W[k,(i,j)]          (PE, bf16)
    m[e,i]      = sum_j bw[e,(i,j)] * atom[src[e],j]    (DVE mult + PE selector-reduce)
    out[t,:]    = sum_{e: tgt[e]=t} m[e,:]              (sorted-edge copy-scatter + reduce)

Feature-major chunked layout: per 512-edge tile, 8 chunks of 128 partitions,
chunk c partition p <-> (j = p//4, i = 4c + p%4).
  - srcT_rep[p, e] = atom[src[e], p//4] via dma_gather transpose (<=512 idxs per
    call -- larger calls overflow the SWDGE ring on HW) from a host-prepped DRAM
    table T[n, 4j+r] = atom[n, j] (bf16).
  - bwT_c = W2_c^T @ bondT_tile (PSUM fp32); most chunks ACT-evacuated to SBUF
    bf16 then DVE-multiplied at 2x; dve_direct_chunks chunks multiplied straight
    from PSUM (1x) to balance ACT vs DVE.
  - mT (32, 512) = sum_c S_c^T @ pT_c (accumulating PSUM); S_c[p,m] = [4c+p%4==m].
  - PE-transpose mT -> edge-major m (128, 32), padded to 64 cols in m_all.
  - Scatter: edges host-sorted by target; processing order = 16 blocks of 1024
    edges by (sorted_pos % 16); same-target edges are consecutive in sorted
    order so each block has unique targets (max in-degree <= 16; dma_scatter_add
    races on duplicate indices WITHIN a call). Block b scatter-adds into DRAM
    copy b//4; the 4 calls per copy are ~8 tiles apart and Tile serializes them
    (WAW), so cross-call duplicates are safe and fencing is free.
  - Final on-device reduce of the NCOPY copies -> out (4096, 32) fp32.
"""
import sys

sys.path.insert(0, "/opt/trn_rl_repo")

import numpy as np

from concourse import bacc, bass, mybir, tile, bass_utils

# problem constants (hardcoded per spec)
B = 8
N = 4096
E = 16384
D = 32          # atom dim
KB = 64         # bond dim
TIL = 512       # edges per pipeline tile
NT = E // TIL   # 32 tiles
CH = 8          # (j,i) chunks per tile
NBLK = 16       # sorted-mod blocks (requires max in-degree <= NBLK)
TPB = E // NBLK  # tokens per block = 1024
NCOPY = 4       # DRAM accumulator copies (block b -> copy b % NCOPY)
GATH = 1        # tiles per dma_gather call (>1 overflows the SWDGE ring on HW)
NPAIR = CH // 2  # chunk pairs per tile (each pair = one 2-bank PSUM tile)
DVE_DIRECT_PAIRS = 1  # pairs whose multiply reads PSUM directly on DVE
MCOPY_ACT = 1   # m_all evac copies per tile done on ACT (rest on DVE)
F32 = mybir.dt.float32
BF16 = mybir.dt.bfloat16
I16 = mybir.dt.int16

_PROGRAM_CACHE = {}

# tunables: SBUF/PSUM pool depths and op-fusion knobs
CFG = dict(
    pair=True,            # fuse chunk pairs into 2-bank PSUM tiles
    bw_bufs=3,            # PSUM bufs for bw tiles (x2 banks if pair)
    mt_bufs=1,            # PSUM bufs for the mT accumulator
    tp_bufs=1,            # PSUM bufs for transpose outputs (0 = share mt pool)
    pt_bufs=8,
    bwsb_bufs=4,
    bt_bufs=8,
    dve_direct_pairs=1,
    dve_direct_chunks=2,
    mcopy_act=1,
    warm_gathers=False,
    nq=1,
)


def _build_program(cfg=None):
    cfg = {**CFG, **(cfg or {})}
    nc = bacc.Bacc("TRN2", target_bir_lowering=False, debug=False, num_devices=B)

    atab_din = nc.dram_tensor("atab", (N, 128), BF16, kind="ExternalInput")
    bondT_d = nc.dram_tensor("bondT", (KB, E), BF16, kind="ExternalInput")
    w2_d = nc.dram_tensor("w2", (KB, CH * 128), BF16, kind="ExternalInput")
    sel_d = nc.dram_tensor("sel", (128, CH * D), BF16, kind="ExternalInput")
    ident_d = nc.dram_tensor("ident", (D, D), F32, kind="ExternalInput")
    srcw_d = nc.dram_tensor("srcw", (128, E // 16), I16, kind="ExternalInput")
    tgtw_d = nc.dram_tensor("tgtw", (128, E // 16), I16, kind="ExternalInput")
    out_d = nc.dram_tensor("out", (N, D), F32, kind="ExternalOutput")

    with tile.TileContext(nc) as tc:
        with tc.tile_pool(name="const", bufs=1) as cp, \
             tc.tile_pool(name="work", bufs=cfg["pt_bufs"]) as wp, \
             tc.tile_pool(name="bwsb", bufs=cfg["bwsb_bufs"]) as bp, \
             tc.tile_pool(name="btp", bufs=cfg["bt_bufs"]) as btp, \
             tc.tile_pool(name="srp", bufs=2) as sp, \
             tc.tile_pool(name="mtev", bufs=2) as mp, \
             tc.tile_pool(name="redu", bufs=2) as rp, \
             tc.tile_pool(name="bwps", bufs=cfg["bw_bufs"], space="PSUM") as bwp, \
             tc.tile_pool(name="mtps", bufs=cfg["mt_bufs"], space="PSUM") as mtp, \
             tc.tile_pool(name="tpps", bufs=max(cfg["tp_bufs"], 1), space="PSUM") as tpp, \
             tc.tile_pool(name="dram", bufs=1, space="DRAM") as dp:
            tp_pool = tpp if cfg["tp_bufs"] > 0 else mtp
            tp_tag = "tp" if cfg["tp_bufs"] > 0 else "mt"

            # ---- one-time setup (srcw first: tile 0's gather needs it) ----
            srcw_sb = cp.tile([128, E // 16], I16)
            nc.sync.dma_start(srcw_sb[:], srcw_d.ap())
            w2_sb = cp.tile([KB, CH * 128], BF16)
            nc.sync.dma_start(w2_sb[:], w2_d.ap())
            sel_sb = cp.tile([128, CH * D], BF16)
            nc.sync.dma_start(sel_sb[:], sel_d.ap())
            ident_sb = cp.tile([D, D], F32)
            nc.sync.dma_start(ident_sb[:], ident_d.ap())
            tgtw_sb = cp.tile([128, E // 16], I16)
            nc.scalar.dma_start(tgtw_sb[:], tgtw_d.ap())

            # gather table T[n, 4j+r] = atom[n, j] (host-prepped bf16 input)
            atab_d = atab_din

            # edge-major messages, token-wrapped: token q at [q%128, q//128, 0:32]
            # (memset emitted after tile 0's gather: Pool SEQ issues in order,
            # and the two big memsets would otherwise delay the first gather)
            m_all = cp.tile([128, E // 128, 64], F32)
            zero_sb = cp.tile([128, (N // 128) * 64], F32)
            copies = [dp.tile([N, 64], F32, name=f"copy{c}") for c in range(NCOPY)]

            def _deferred_setup():
                nc.gpsimd.memset(m_all[:], 0.0)
                nc.gpsimd.memset(zero_sb[:], 0.0)
                for c in range(NCOPY):
                    nc.scalar.dma_start(
                        copies[c][:].rearrange("(p g) j -> p (g j)", p=128),
                        zero_sb[:],
                    )

            # ---- main pipeline ----
            for t in range(NT):
                esl = slice(t * TIL, (t + 1) * TIL)

                bt_sb = btp.tile([KB, TIL], BF16, tag="bt")
                bt_eng = nc.scalar if cfg.get("bt_on_act") else nc.sync
                bt_eng.dma_start(bt_sb[:], bondT_d.ap()[:, esl])

                # optionally: first two gathers cover 1 tile each (fast start)
                nwarm = 2 if cfg.get("warm_gathers", True) else 0
                if t < nwarm or (t - nwarm) % GATH == 0:
                    ng = 1 if t < nwarm else min(GATH, NT - t)
                    srep = sp.tile([128, 1, GATH * TIL], BF16, tag="srep")
                    nidx = ng * TIL
                    nc.gpsimd.dma_gather(
                        out_ap=srep[:, :, :nidx],
                        in_ap=atab_d.ap(),
                        idxs_ap=srcw_sb[:, t * (TIL // 16):(t + ng) * (TIL // 16)],
                        num_idxs=nidx,
                        num_idxs_reg=nidx,
                        elem_size=128,
                        transpose=True,
                    )
                    srep_base = t
                ssl = slice((t - srep_base) * TIL, (t - srep_base + 1) * TIL)

                if t == 0:
                    _deferred_setup()

                mt_ps = mtp.tile([D, TIL], F32, tag="mt")
                if cfg["pair"]:
                    # srep broadcast over a chunk pair: [2 (step 0), TIL (step 1)]
                    srep_pair = srep[:, 0:1, ssl].to_broadcast([128, 2, TIL])
                    for pr in range(NPAIR):
                        bw_ps = bwp.tile([128, 2, TIL], F32, tag="bw")
                        for h in range(2):
                            c = 2 * pr + h
                            nc.tensor.matmul(
                                out=bw_ps[:, h, :],
                                lhsT=w2_sb[:, c * 128:(c + 1) * 128],
                                rhs=bt_sb[:],
                                start=True, stop=True,
                            )
                        pt_sb = wp.tile([128, 2, TIL], BF16, tag="pt")
                        if pr < cfg["dve_direct_pairs"]:
                            nc.vector.tensor_tensor(
                                out=pt_sb[:], in0=bw_ps[:], in1=srep_pair,
                                op=mybir.AluOpType.mult,
                            )
                        else:
                            bw_sb = bp.tile([128, 2, TIL], BF16, tag="bwsb")
                            nc.scalar.copy(bw_sb[:], bw_ps[:])
                            nc.vector.tensor_tensor(
                                out=pt_sb[:], in0=bw_sb[:], in1=srep_pair,
                                op=mybir.AluOpType.mult,
                            )
                        for h in range(2):
                            c = 2 * pr + h
                            nc.tensor.matmul(
                                out=mt_ps[:],
                                lhsT=sel_sb[:, c * D:(c + 1) * D],
                                rhs=pt_sb[:, h, :],
                                start=(c == 0), stop=(c == CH - 1),
                            )
                else:
                    ndir = cfg.get("dve_direct_chunks", 2 * cfg["dve_direct_pairs"])
                    for c in range(CH):
                        bw_ps = bwp.tile([128, TIL], F32, tag="bw")
                        nc.tensor.matmul(
                            out=bw_ps[:],
                            lhsT=w2_sb[:, c * 128:(c + 1) * 128],
                            rhs=bt_sb[:],
                            start=True, stop=True,
                        )
                        pt_sb = wp.tile([128, TIL], BF16, tag="pt")
                        if c < ndir:
                            nc.vector.tensor_tensor(
                                out=pt_sb[:], in0=bw_ps[:], in1=srep[:, 0, ssl],
                                op=mybir.AluOpType.mult,
                            )
                        else:
                            bw_sb = bp.tile([128, TIL], BF16, tag="bwsb")
                            nc.scalar.copy(bw_sb[:], bw_ps[:])
                            eng = (nc.gpsimd if c >= CH - cfg.get("gp_chunks", 0)
                                   else nc.vector)
                            eng.tensor_tensor(
                                out=pt_sb[:], in0=bw_sb[:], in1=srep[:, 0, ssl],
                                op=mybir.AluOpType.mult,
                            )
                        nc.tensor.matmul(
                            out=mt_ps[:],
                            lhsT=sel_sb[:, c * D:(c + 1) * D],
                            rhs=pt_sb[:],
                            start=(c == 0), stop=(c == CH - 1),
                        )

                mt_sb = mp.tile([D, TIL], F32, tag="mtsb")
                if cfg.get("mt_evac_dve"):
                    nc.vector.tensor_copy(mt_sb[:], mt_ps[:])
                else:
                    nc.scalar.copy(mt_sb[:], mt_ps[:])

                for q in range(TIL // 128):
                    tp_ps = tp_pool.tile([128, D], F32, tag=tp_tag, name="tp_ps")
                    nc.tensor.transpose(
                        tp_ps[:], mt_sb[:, q * 128:(q + 1) * 128], ident_sb[:]
                    )
                    slot = t * (TIL // 128) + q
                    if q < cfg["mcopy_act"]:
                        nc.scalar.copy(m_all[:, slot, 0:D], tp_ps[:])
                    else:
                        nc.vector.tensor_copy(m_all[:, slot, 0:D], tp_ps[:])

                # one block (= 2 tiles = 1024 tokens) completed -> scatter it
                if t % 2 == 1:
                    blk = t // 2
                    nc.gpsimd.dma_scatter_add(
                        out_ap=copies[blk // (NBLK // NCOPY)][:],
                        in_ap=m_all[:, blk * (TPB // 128):(blk + 1) * (TPB // 128), :],
                        idxs_ap=tgtw_sb[:, blk * (TPB // 16):(blk + 1) * (TPB // 16)],
                        num_idxs=TPB,
                        num_idxs_reg=TPB,
                        elem_size=64,
                    )

            # ---- final reduce of the copies, split over node ranges ----
            # copy tensors and out use p-major node layout: row n = 32*p + g
            NQ = cfg.get("nq", 4)
            GQ = (N // 128) // NQ  # node-groups per range
            for q in range(NQ):
                acc = rp.tile([128, GQ * D], F32, tag="acc")
                for c in range(NCOPY):
                    csl = rp.tile([128, GQ, D], F32, tag="csl")
                    nc.sync.dma_start(
                        csl[:],
                        copies[c][:].rearrange("(p g) j -> p g j", p=128)
                        [:, q * GQ:(q + 1) * GQ, 0:D],
                    )
                    if c == 0:
                        nc.vector.tensor_copy(
                            acc[:], csl[:].rearrange("p g j -> p (g j)")
                        )
                    else:
                        nc.vector.tensor_tensor(
                            out=acc[:], in0=acc[:],
                            in1=csl[:].rearrange("p g j -> p (g j)"),
                            op=mybir.AluOpType.add,
                        )
                nc.sync.dma_start(
                    out_d.ap().rearrange("(p g) j -> p g j", p=128)
                    [:, q * GQ:(q + 1) * GQ, :].rearrange("p g j -> p (g j)"),
                    acc[:],
                )

    nc.compile()
    return nc


def _host_prep(atom_state, bond_state, bond_transform, connectivity):
    """Build per-core input maps. Pure layout / index-metadata / dtype prep."""
    import ml_dtypes

    W = np.asarray(bond_transform, dtype=np.float32)  # (KB, D*D)

    # W2[k, c*128 + p] = W[k, (4c + p%4)*D + p//4]   (i = 4c + p%4, j = p//4)
    p = np.arange(128)
    cc = np.arange(CH)
    i_idx = 4 * cc[:, None] + (p % 4)[None, :]   # (CH, 128)
    j_idx = np.broadcast_to((p // 4)[None, :], (CH, 128))
    w2 = W[:, (i_idx * D + j_idx).reshape(-1)].astype(ml_dtypes.bfloat16)

    # selectors S_c[p, m] = [4c + p%4 == m]
    sel = np.zeros((128, CH * D), dtype=np.float32)
    for c in range(CH):
        sel[p, c * D + 4 * c + (p % 4)] = 1.0
    sel_bf = sel.astype(ml_dtypes.bfloat16)

    ident = np.eye(D, dtype=np.float32)

    in_maps = []
    for b in range(B):
        src = np.asarray(connectivity[b, :, 0], dtype=np.int64)
        tgt = np.asarray(connectivity[b, :, 1], dtype=np.int64)
        order = np.argsort(tgt, kind="stable")
        deg = np.bincount(tgt, minlength=N).max()
        if deg > NBLK:
            raise ValueError(f"max in-degree {deg} exceeds {NBLK}")
        # processing order: blocks by sorted_pos % NBLK
        proc = np.concatenate([order[c::NBLK] for c in range(NBLK)])
        srcp = src[proc].astype(np.int16)
        tgtp = tgt[proc].astype(np.int16)

        bondT = np.ascontiguousarray(
            np.asarray(bond_state[b], dtype=np.float32).T[:, proc]
        ).astype(ml_dtypes.bfloat16)  # (KB, E)

        # wrapped idx tables: idxs[p, s] = vals[16*s + p%16], tiled to 128 partitions
        def wrap16(vals):
            w = vals.reshape(-1, 16).T  # (16, E//16)
            return np.ascontiguousarray(np.tile(w, (8, 1)), dtype=np.int16)

        # gather table T[n, 4j+r] = atom[n, j]
        atab = np.repeat(
            np.asarray(atom_state[b], dtype=np.float32), 4, axis=1
        ).astype(ml_dtypes.bfloat16)

        in_maps.append({
            "atab": np.ascontiguousarray(atab),
            "bondT": bondT,
            "w2": w2,
            "sel": sel_bf,
            "ident": ident,
            "srcw": wrap16(srcp),
            "tgtw": wrap16(tgtp),
        })
    return in_maps


def kernel(atom_state, bond_state, bond_transform, connectivity):
    if "nc" not in _PROGRAM_CACHE:
        _PROGRAM_CACHE["nc"] = _build_program()
    nc = _PROGRAM_CACHE["nc"]

    in_maps = _host_prep(atom_state, bond_state, bond_transform, connectivity)
    res = bass_utils.run_bass_kernel_spmd(nc, in_maps, list(range(B)))
    out = np.stack([res.results[b]["out"] for b in range(B)], axis=0)
    return out.astype(np.float32)



# revision 19
# speedup vs baseline: 1.2022x; 1.2022x over previous
"""Trainium2 Bass kernel for nn_BondMatrixMessage (GNN bond-matrix message passing).

Per batch b (one NeuronCore each, B=8 => 8 cores):
    bw[e,(i,j)] = sum_k bond[e,k] * W[k,(i,j)]          (PE, bf16)
    m[e,i]      = sum_j bw[e,(i,j)] * atom[src[e],j]    (DVE mult + tiny PE reduce)
    out[t,:]    = sum_{e: tgt[e]=t} m[e,:]              (sorted-edge scatter + reduce)

Feature-major chunked layout: per 512-edge tile, 8 chunks of 128 partitions,
chunk c partition p <-> (j = p//4, i = 4c + p%4).
  - srcT_rep[p, e] = atom[src[e], p//4] via dma_gather transpose from a
    host-prepped DRAM table T[n, 4j+r] = atom[n, j] (bf16). Gathers batched
    4 tiles per call (SWDGE ring enlarged to 4096 descriptors) and PREFETCHED
    two iterations ahead, emitted at the head of Pool's queue so they never
    sit behind a scatter.
  - bwT_c = W2_c^T @ bondT_tile (PSUM fp32, pair-fused 2-bank tiles); evac+mult
    split across ACT copy / Pool copy + DVE tensor_tensor per CFG pair plan.
  - m (128e, 32) = per chunk: matmul(lhsT=pt_c slab (128,128), rhs=sel4 (128,4))
    writing disjoint 4-col slices of one 1-bank PSUM tile -> edge-major m
    directly (4-col moving operand => ~6ns/matmul; no transposes, no selector
    sweeps). One copy evacuates (128,4,32) per tile into m_all (engine
    alternates Pool/DVE).
  - PE stream software-pipelined one tile: bw(t) emitted before mproj(t-1) so
    mproj's wait on pt never stalls the next tile's bw matmuls.
  - Scatter: edges host-sorted by target; 16 blocks of 1024 edges by
    (sorted_pos % 16); block b scatter-adds into DRAM copy b//4 (WAW-serialized
    per copy). m_all pad cols zeroed by one Pool memset (no DMA jam); copy
    zero-fills staggered across the timeline on the ACT HWDGE queue. Copies
    are partially reduced EARLY (copy c is complete at tile 8c+7) so only
    copy 3's reduce is on the tail; copy 0 is DMA-loaded straight into acc.
"""
import sys

sys.path.insert(0, "/opt/trn_rl_repo")

import numpy as np

from concourse import bacc, bass, mybir, tile, bass_utils

# problem constants (hardcoded per spec)
B = 8
N = 4096
E = 16384
D = 32          # atom dim
KB = 64         # bond dim
TIL = 512       # edges per pipeline tile
NT = E // TIL   # 32 tiles
CH = 8          # (j,i) chunks per tile
NPAIR = CH // 2
NBLK = 16       # sorted-mod blocks (requires max in-degree <= NBLK)
TPB = E // NBLK  # tokens per block = 1024
NCOPY = 4       # DRAM accumulator copies
BLK2COPY = [0]*6 + [1]*5 + [2]*4 + [3]*1  # block -> copy (last copy: 1 block)
# copy c complete once its last block's scatter lands (after mproj of tile:)
COPY_DONE_TILE = [11, 21, 29, 31]
F32 = mybir.dt.float32
BF16 = mybir.dt.bfloat16
I16 = mybir.dt.int16

_PROGRAM_CACHE = {}

CFG = dict(
    # evac+mult plan per chunk-pair, alternating by tile parity:
    #   'A': ACT pair evac + DVE pair mult (ACT ~1038, DVE ~594)
    #   'D': DVE direct-from-PSUM pair mult   (DVE ~1192)
    #   'P': Pool pair evac + DVE pair mult   (Pool ~900, DVE ~594)
    #   'H': ACT half + Pool half evac, 2 DVE half mults
    plans=(("A", "A", "A", "D"),),  # per-tile pair plan cycle
    plan_even=("A", "A", "P", "D"),
    plan_odd=("A", "A", "A", "D"),
    plan_tail=("A", "A", "A", "A"),  # last 2 tiles: minimize DVE latency
    plan_head=None,
    mproj_pos=3,    # emit mproj(t-1) after this many pairs of tile t
    mevac=("dve",),  # m_ps evac engine, cycled by tile index
    bw_bufs=3,
    pt_bufs=6,
    bt_bufs=8,
    srp_bufs=3,
    mps_bufs=2,
    warm_pe=6,      # dummy 512-col matmuls at setup to ramp PE to 2.4 GHz
    nq=2,           # node-range splits in the copy reduction
    gath=1,         # tiles per steady-state dma_gather call (>1 overflows SWDGE ring on HW)
    prefetch=2,     # iterations ahead to emit each gather
    ring=16384,     # dynamic_dma_scratch_size (default; HW ring holds 1024 descs)
)


def _build_program(cfg=None):
    cfg = {**CFG, **(cfg or {})}
    nc = bacc.Bacc("TRN2", target_bir_lowering=False, debug=False, num_devices=B,
                   dynamic_dma_scratch_size=cfg["ring"])

    atab_din = nc.dram_tensor("atab", (N, 128), BF16, kind="ExternalInput")
    bondT_d = nc.dram_tensor("bondT", (KB, E), BF16, kind="ExternalInput")
    w2_d = nc.dram_tensor("w2", (KB, CH * 128), BF16, kind="ExternalInput")
    sel4_d = nc.dram_tensor("sel4", (128, 4), BF16, kind="ExternalInput")
    srcw_d = nc.dram_tensor("srcw", (128, E // 16), I16, kind="ExternalInput")
    tgtw_d = nc.dram_tensor("tgtw", (128, E // 16), I16, kind="ExternalInput")
    out_d = nc.dram_tensor("out", (N, D), F32, kind="ExternalOutput")

    GATH = cfg["gath"]
    PRE = cfg["prefetch"]
    # gather ranges (t0, ntiles); emission iteration = max(0, t0 - PRE)
    granges = [(0, 1), (1, 1), (2, 2)] if GATH > 1 else [(0, 1), (1, 1), (2, 1), (3, 1)]
    t0 = 4
    while t0 < NT:
        granges.append((t0, min(GATH, NT - t0)))
        t0 += GATH
    emit_at = {}
    for t0, ng in granges:
        emit_at.setdefault(max(0, t0 - PRE), []).append((t0, ng))
    # staggered copy-zero DMAs: copy c needed by tile 8c+1
    zero_at = {0: [0], 2: [1], 8: [2], 16: [3]}
    NQ = cfg["nq"]
    GQ = (N // 128) // NQ  # node-groups per range

    with tile.TileContext(nc) as tc:
        with tc.tile_pool(name="const", bufs=1) as cp, \
             tc.tile_pool(name="work", bufs=cfg["pt_bufs"]) as wp, \
             tc.tile_pool(name="bwsb", bufs=cfg.get("bwsb_bufs", 6)) as bp, \
             tc.tile_pool(name="btp", bufs=cfg["bt_bufs"]) as btp, \
             tc.tile_pool(name="srp", bufs=cfg["srp_bufs"]) as sp, \
             tc.tile_pool(name="redu", bufs=2) as rp, \
             tc.tile_pool(name="bwps", bufs=cfg["bw_bufs"], space="PSUM") as bwp, \
             tc.tile_pool(name="mps", bufs=cfg["mps_bufs"], space="PSUM") as mpp, \
             tc.tile_pool(name="dram", bufs=1, space="DRAM") as dp:

            # ---- one-time setup (srcw first: tile 0's gather needs it) ----
            srcw_sb = cp.tile([128, E // 16], I16)
            nc.sync.dma_start(srcw_sb[:], srcw_d.ap())
            w2_sb = cp.tile([KB, CH * 128], BF16)
            nc.sync.dma_start(w2_sb[:], w2_d.ap())
            sel4_sb = cp.tile([128, 4], BF16)
            tgtw_sb = cp.tile([128, E // 16], I16)

            atab_d = atab_din

            # edge-major messages, token-wrapped: token q at [q%128, q//128, 0:32]
            m_all = cp.tile([128, E // 128, 64], F32)
            zero_sb = cp.tile([128, (N // 128) * 64], F32)
            copies = [dp.tile([N, 64], F32, name=f"copy{c}") for c in range(NCOPY)]

            # warm the ACT activation table during idle setup
            warm_act = cp.tile([128, 4], BF16)
            nc.scalar.copy(warm_act[:], sel4_sb[:])

            def _zero_copy(c):
                nc.scalar.dma_start(
                    copies[c][:].rearrange("(p g) j -> p (g j)", p=128),
                    zero_sb[:],
                )

            def _deferred_setup():
                nc.sync.dma_start(sel4_sb[:], sel4_d.ap())
                nc.scalar.dma_start(tgtw_sb[:], tgtw_d.ap())
                # PE clock warmup: back-to-back dummy matmuls
                if cfg["warm_pe"]:
                    wps = bwp.tile([128, 2, TIL], F32, tag="bw")
                    for i in range(cfg["warm_pe"]):
                        nc.tensor.matmul(
                            out=wps[:, i % 2, :],
                            lhsT=w2_sb[:, 0:128],
                            rhs=w2_sb[:, 0:512],
                            start=True, stop=True,
                        )
                nc.gpsimd.memset(zero_sb[:], 0.0)
                # zero scatter pad cols of m_all for blocks 0-3 (tiles 0-7)
                nc.gpsimd.memset(m_all[:, 0:32, 32:64], 0.0)

            # early partial reduction state
            acc = [rp.tile([128, GQ, D], F32, tag=f"acc{q}", name=f"acc{q}")
                   for q in range(NQ)]

            def _reduce_copy(c):
                """Fold copies[c] into acc (copy c complete after tile 8c+7).

                Loads alternate between the SP and ACT HWDGE queues so the
                final copy's node-range chains run concurrently."""
                for q in range(NQ):
                    eng = (nc.sync if (q % 2 == 0 or c < NCOPY - 1)
                           else nc.scalar)
                    view = copies[c][:].rearrange("(p g) j -> p g j", p=128)[
                        :, q * GQ:(q + 1) * GQ, 0:D]
                    if c == 0:
                        eng.dma_start(acc[q][:], view)
                    else:
                        csl = rp.tile([128, GQ, D], F32, tag=f"csl{q % 2}")
                        eng.dma_start(csl[:], view)
                        add_eng = (nc.gpsimd if (c < NCOPY - 1 and
                                   cfg.get("acc_pool", True)) else nc.vector)
                        add_eng.tensor_tensor(
                            out=acc[q][:], in0=acc[q][:], in1=csl[:],
                            op=mybir.AluOpType.add,
                        )

            srep_info = {}   # tile -> (srep tile, base)
            prev = None      # (pt tiles for tile t-1, t-1)

            def _emit_gathers(t):
                for t0, ng in emit_at.get(t, []):
                    srep = sp.tile([128, 1, GATH * TIL], BF16, tag="srep")
                    nidx = ng * TIL
                    nc.gpsimd.dma_gather(
                        out_ap=srep[:, :, :nidx],
                        in_ap=atab_d.ap(),
                        idxs_ap=srcw_sb[:, t0 * (TIL // 16):(t0 + ng) * (TIL // 16)],
                        num_idxs=nidx,
                        num_idxs_reg=nidx,
                        elem_size=128,
                        transpose=True,
                    )
                    for tt in range(t0, t0 + ng):
                        srep_info[tt] = (srep, t0)

            def _emit_mproj(pts, tt):
                """Tiny-matmul projection + evac + scatter for tile tt."""
                m_ps = mpp.tile([128, 4, D], F32, tag="mps")
                for g in range(4):
                    gsl = slice(g * 128, (g + 1) * 128)
                    for c in range(CH):
                        nc.tensor.matmul(
                            out=m_ps[:, g, 4 * c:4 * c + 4],
                            lhsT=pts[c // 2][:, c % 2, gsl],
                            rhs=sel4_sb[:],
                            start=True, stop=True,
                        )
                meng = cfg["mevac"][tt % len(cfg["mevac"])]
                dst = m_all[:, tt * 4:(tt + 1) * 4, 0:D]
                if meng == "dma":
                    nc.sync.dma_start(dst, m_ps[:])
                elif meng == "act":
                    nc.scalar.copy(dst, m_ps[:])
                else:
                    nc.vector.tensor_copy(dst, m_ps[:])
                if tt >= NT - 2 and cfg.get("half_tail", True):
                    # last block: per-tile half scatters (targets within a
                    # block are unique, so the halves write disjoint rows)
                    blk = NBLK - 1
                    half = tt % 2
                    hsl = slice(blk * (TPB // 16) + half * (TPB // 32),
                                blk * (TPB // 16) + (half + 1) * (TPB // 32))
                    nc.gpsimd.dma_scatter_add(
                        out_ap=copies[NCOPY - 1][:],
                        in_ap=m_all[:, tt * 4:(tt + 1) * 4, :],
                        idxs_ap=tgtw_sb[:, hsl],
                        num_idxs=TPB // 2,
                        num_idxs_reg=TPB // 2,
                        elem_size=64,
                    )
                elif tt % 2 == 1:
                    blk = tt // 2
                    nc.gpsimd.dma_scatter_add(
                        out_ap=copies[BLK2COPY[blk]][:],
                        in_ap=m_all[:, blk * (TPB // 128):(blk + 1) * (TPB // 128), :],
                        idxs_ap=tgtw_sb[:, blk * (TPB // 16):(blk + 1) * (TPB // 16)],
                        num_idxs=TPB,
                        num_idxs_reg=TPB,
                        elem_size=64,
                    )

            # ---- main pipeline ----
            for t in range(NT):
                _emit_gathers(t)

                bt_sb = btp.tile([KB, TIL], BF16, tag="bt")
                nc.sync.dma_start(bt_sb[:], bondT_d.ap()[:, t * TIL:(t + 1) * TIL])

                if t == 0:
                    _deferred_setup()
                for c in zero_at.get(t, []):
                    _zero_copy(c)
                if t == 3:
                    nc.gpsimd.memset(m_all[:, 32:128, 32:64], 0.0)

                srep, srep_base = srep_info[t]
                ssl = slice((t - srep_base) * TIL, (t - srep_base + 1) * TIL)
                srep_half = srep[:, 0, ssl]
                srep_pair = srep[:, 0:1, ssl].to_broadcast([128, 2, TIL])

                # bw matmuls + evac/mult for tile t (PE: before mproj(t-1))
                if t < 2 and cfg.get("plan_head"):
                    plan = cfg["plan_head"]
                elif t >= NT - 2 and cfg.get("plan_tail"):
                    plan = cfg["plan_tail"]
                elif cfg.get("plans"):
                    plan = cfg["plans"][t % len(cfg["plans"])]
                else:
                    plan = cfg["plan_even"] if t % 2 == 0 else cfg["plan_odd"]
                pts = []
                for pr in range(NPAIR):
                    bw_ps = bwp.tile([128, 2, TIL], F32, tag="bw")
                    for h in range(2):
                        c = 2 * pr + h
                        nc.tensor.matmul(
                            out=bw_ps[:, h, :],
                            lhsT=w2_sb[:, c * 128:(c + 1) * 128],
                            rhs=bt_sb[:],
                            start=True, stop=True,
                        )
                    pt_sb = wp.tile([128, 2, TIL], BF16, tag="pt")
                    mode = plan[pr]
                    if mode == "D":
                        # DVE direct mult from PSUM (1x)
                        nc.vector.tensor_tensor(
                            out=pt_sb[:], in0=bw_ps[:], in1=srep_pair,
                            op=mybir.AluOpType.mult,
                        )
                    elif mode == "A":
                        # ACT pair evac + DVE pair mult (GPSIMD cannot
                        # access PSUM on HW, so evac is ACT or DVE only)
                        bw_sb = bp.tile([128, 2, TIL], BF16, tag="bwsb")
                        nc.scalar.copy(bw_sb[:], bw_ps[:])
                        nc.vector.tensor_tensor(
                            out=pt_sb[:], in0=bw_sb[:], in1=srep_pair,
                            op=mybir.AluOpType.mult,
                        )
                    elif mode == "Y":
                        # half 0: ACT evac + DVE mult; half 1: DVE direct
                        bw_sb = bp.tile([128, 2, TIL], BF16, tag="bwsb")
                        nc.scalar.copy(bw_sb[:, 0, :], bw_ps[:, 0, :])
                        nc.vector.tensor_tensor(
                            out=pt_sb[:, 0, :], in0=bw_sb[:, 0, :],
                            in1=srep_half, op=mybir.AluOpType.mult,
                        )
                        nc.vector.tensor_tensor(
                            out=pt_sb[:, 1, :], in0=bw_ps[:, 1, :],
                            in1=srep_half, op=mybir.AluOpType.mult,
                        )
                    else:
                        raise ValueError(mode)
                    pts.append(pt_sb)
                    # mproj for tile t-1 (software-pipelined PE stream),
                    # emitted mid-tile so m_ps/m-evac land early on DVE
                    if len(pts) == cfg["mproj_pos"] and prev is not None:
                        _emit_mproj(prev[0], prev[1])
                        prev = None

                if prev is not None:
                    _emit_mproj(prev[0], prev[1])
                prev = (pts, t)

                # early copy reduction (copy c done at COPY_DONE_TILE[c])
                for c in range(NCOPY - 1):
                    if t == COPY_DONE_TILE[c] + 2:
                        _reduce_copy(c)

            # ---- drain: mproj(31), last scatter, final reduce ----
            _emit_mproj(prev[0], prev[1])
            _reduce_copy(NCOPY - 1)
            for q in range(NQ):
                nc.sync.dma_start(
                    out_d.ap().rearrange("(p g) j -> p g j", p=128)
                    [:, q * GQ:(q + 1) * GQ, :],
                    acc[q][:],
                )

    nc.compile()
    return nc


def _host_prep(atom_state, bond_state, bond_transform, connectivity):
    """Build per-core input maps. Pure layout / index-metadata / dtype prep."""
    import ml_dtypes

    W = np.asarray(bond_transform, dtype=np.float32)  # (KB, D*D)

    # W2[k, c*128 + p] = W[k, (4c + p%4)*D + p//4]   (i = 4c + p%4, j = p//4)
    p = np.arange(128)
    cc = np.arange(CH)
    i_idx = 4 * cc[:, None] + (p % 4)[None, :]   # (CH, 128)
    j_idx = np.broadcast_to((p // 4)[None, :], (CH, 128))
    w2 = W[:, (i_idx * D + j_idx).reshape(-1)].astype(ml_dtypes.bfloat16)

    # sel4[p, r] = [p % 4 == r]
    sel4 = np.zeros((128, 4), dtype=np.float32)
    sel4[p, p % 4] = 1.0
    sel4 = sel4.astype(ml_dtypes.bfloat16)

    in_maps = []
    for b in range(B):
        src = np.asarray(connectivity[b, :, 0], dtype=np.int64)
        tgt = np.asarray(connectivity[b, :, 1], dtype=np.int64)
        order = np.argsort(tgt, kind="stable")
        deg = np.bincount(tgt, minlength=N).max()
        if deg > NBLK:
            raise ValueError(f"max in-degree {deg} exceeds {NBLK}")
        # processing order: blocks by sorted_pos % NBLK
        proc = np.concatenate([order[c::NBLK] for c in range(NBLK)])
        srcp = src[proc].astype(np.int16)
        tgtp = tgt[proc].astype(np.int16)

        bondT = np.ascontiguousarray(
            np.asarray(bond_state[b], dtype=np.float32).T[:, proc]
        ).astype(ml_dtypes.bfloat16)  # (KB, E)

        # wrapped idx tables: idxs[p, s] = vals[16*s + p%16], tiled to 128 partitions
        def wrap16(vals):
            w_ = vals.reshape(-1, 16).T  # (16, E//16)
            return np.ascontiguousarray(np.tile(w_, (8, 1)), dtype=np.int16)

        # gather table T[n, 4j+r] = atom[n, j]
        atab = np.repeat(
            np.asarray(atom_state[b], dtype=np.float32), 4, axis=1
        ).astype(ml_dtypes.bfloat16)

        in_maps.append({
            "atab": np.ascontiguousarray(atab),
            "bondT": bondT,
            "w2": w2,
            "sel4": sel4,
            "srcw": wrap16(srcp),
            "tgtw": wrap16(tgtp),
        })
    return in_maps


def kernel(atom_state, bond_state, bond_transform, connectivity):
    if "nc" not in _PROGRAM_CACHE:
        _PROGRAM_CACHE["nc"] = _build_program()
    nc = _PROGRAM_CACHE["nc"]

    in_maps = _host_prep(atom_state, bond_state, bond_transform, connectivity)
    res = bass_utils.run_bass_kernel_spmd(nc, in_maps, list(range(B)))
    out = np.stack([res.results[b]["out"] for b in range(B)], axis=0)
    return out.astype(np.float32)


# revision 21
# speedup vs baseline: 1.2117x; 1.0079x over previous
"""Trainium2 Bass kernel for nn_BondMatrixMessage (GNN bond-matrix message passing).

Per batch b (one NeuronCore each, B=8 => 8 cores):
    bw[e,(i,j)] = sum_k bond[e,k] * W[k,(i,j)]          (PE, bf16)
    m[e,i]      = sum_j bw[e,(i,j)] * atom[src[e],j]    (DVE mult + tiny PE reduce)
    out[t,:]    = sum_{e: tgt[e]=t} m[e,:]              (sorted-edge scatter + reduce)

Feature-major chunked layout: per 512-edge tile, 8 chunks of 128 partitions,
chunk c partition p <-> (j = p//4, i = 4c + p%4).
  - srcT_rep[p, e] = atom[src[e], p//4] via dma_gather transpose from a
    host-prepped DRAM table T[n, 4j+r] = atom[n, j] (bf16); one 512-idx call
    per tile (bigger calls overflow the HW SWDGE ring), PREFETCHED two
    iterations ahead and emitted at the head of Pool's queue so it never
    sits behind a scatter.
  - bwT_c = W2_c^T @ bondT_tile (PSUM fp32, pair-fused 2-bank tiles). Only
    ACT and DVE may read PSUM on HW (GPSIMD may not): per tile, 3 pairs are
    ACT-evacuated to bf16 SBUF then DVE-multiplied at 2x, 1 pair is
    DVE-multiplied straight from PSUM at 1x (CFG plans).
  - m (128e, 32) = per chunk: matmul(lhsT=pt_c slab (128,128), rhs=sel4 (128,4))
    writing disjoint 4-col slices of one 1-bank PSUM tile -> edge-major m
    directly (4-col moving operand => ~6ns/matmul; no transposes, no selector
    sweeps). One copy evacuates (128,4,32) per tile into m_all (engine
    alternates Pool/DVE).
  - PE stream software-pipelined one tile: bw(t) emitted before mproj(t-1) so
    mproj's wait on pt never stalls the next tile's bw matmuls.
  - Scatter: edges host-sorted by target; 16 blocks of 1024 edges by
    (sorted_pos % 16); block b scatter-adds into DRAM copy BLK2COPY[b]
    ({6,5,4,1} blocks per copy, WAW-serialized per copy; the last block is
    scattered as two per-tile 512-token halves -- targets within a block are
    unique so the halves hit disjoint rows). m_all pad cols zeroed by Pool
    memsets; copy zero-fills staggered on the ACT HWDGE queue. Copies are
    partially reduced EARLY (copy c complete at COPY_DONE_TILE[c]; adds on
    Pool) so only copy 3's single block is on the tail; copy 0 is DMA-loaded
    straight into acc. m_ps evac engine, reduction queue split, pair plans,
    and gather prefetch distance are CFG knobs tuned against TimelineSim.
"""
import sys

sys.path.insert(0, "/opt/trn_rl_repo")

import numpy as np

from concourse import bacc, bass, mybir, tile, bass_utils

# problem constants (hardcoded per spec)
B = 8
N = 4096
E = 16384
D = 32          # atom dim
KB = 64         # bond dim
TIL = 512       # edges per pipeline tile
NT = E // TIL   # 32 tiles
CH = 8          # (j,i) chunks per tile
NPAIR = CH // 2
NBLK = 16       # sorted-mod blocks (requires max in-degree <= NBLK)
TPB = E // NBLK  # tokens per block = 1024
NCOPY = 4       # DRAM accumulator copies
BLK2COPY = [0]*6 + [1]*5 + [2]*4 + [3]*1  # block -> copy (last copy: 1 block)
# copy c complete once its last block's scatter lands (after mproj of tile:)
COPY_DONE_TILE = [11, 21, 29, 31]
F32 = mybir.dt.float32
BF16 = mybir.dt.bfloat16
I16 = mybir.dt.int16

_PROGRAM_CACHE = {}

CFG = dict(
    # evac+mult plan per chunk-pair, alternating by tile parity:
    #   'A': ACT pair evac + DVE pair mult (ACT ~1038, DVE ~594)
    #   'D': DVE direct-from-PSUM pair mult   (DVE ~1192)
    #   'P': Pool pair evac + DVE pair mult   (Pool ~900, DVE ~594)
    #   'H': ACT half + Pool half evac, 2 DVE half mults
    plans=(("A", "A", "A", "D"),),  # per-tile pair plan cycle
    plan_even=("A", "A", "P", "D"),
    plan_odd=("A", "A", "A", "D"),
    plan_tail=("A", "A", "A", "A"),  # last 2 tiles: minimize DVE latency
    plan_head=None,
    mproj_pos=3,    # emit mproj(t-1) after this many pairs of tile t
    mevac=("dve",),  # m_ps evac engine, cycled by tile index
    bw_bufs=3,
    pt_bufs=6,
    bt_bufs=8,
    srp_bufs=3,
    mps_bufs=2,
    warm_pe=1,      # dummy 512-col matmul at setup to start the PE clock ramp
    nq=2,           # node-range splits in the copy reduction
    gath=1,         # tiles per steady-state dma_gather call (>1 overflows SWDGE ring on HW)
    prefetch=2,     # iterations ahead to emit each gather
    ring=16384,     # dynamic_dma_scratch_size (default; HW ring holds 1024 descs)
)


def _build_program(cfg=None):
    cfg = {**CFG, **(cfg or {})}
    nc = bacc.Bacc("TRN2", target_bir_lowering=False, debug=False, num_devices=B,
                   dynamic_dma_scratch_size=cfg["ring"])

    atab_din = nc.dram_tensor("atab", (N, 128), BF16, kind="ExternalInput")
    bondT_d = nc.dram_tensor("bondT", (KB, E), BF16, kind="ExternalInput")
    w2_d = nc.dram_tensor("w2", (KB, CH * 128), BF16, kind="ExternalInput")
    sel4_d = nc.dram_tensor("sel4", (128, 4), BF16, kind="ExternalInput")
    srcw_d = nc.dram_tensor("srcw", (128, E // 16), I16, kind="ExternalInput")
    tgtw_d = nc.dram_tensor("tgtw", (128, E // 16), I16, kind="ExternalInput")
    out_d = nc.dram_tensor("out", (N, D), F32, kind="ExternalOutput")

    GATH = cfg["gath"]
    PRE = cfg["prefetch"]
    # gather ranges (t0, ntiles); emission iteration = max(0, t0 - PRE)
    granges = [(0, 1), (1, 1), (2, 2)] if GATH > 1 else [(0, 1), (1, 1), (2, 1), (3, 1)]
    t0 = 4
    while t0 < NT:
        granges.append((t0, min(GATH, NT - t0)))
        t0 += GATH
    emit_at = {}
    for t0, ng in granges:
        emit_at.setdefault(max(0, t0 - PRE), []).append((t0, ng))
    # staggered copy-zero DMAs: copy c needed by tile 8c+1
    zero_at = {0: [0], 2: [1], 8: [2], 16: [3]}
    NQ = cfg["nq"]
    GQ = (N // 128) // NQ  # node-groups per range

    with tile.TileContext(nc) as tc:
        with tc.tile_pool(name="const", bufs=1) as cp, \
             tc.tile_pool(name="work", bufs=cfg["pt_bufs"]) as wp, \
             tc.tile_pool(name="bwsb", bufs=cfg.get("bwsb_bufs", 6)) as bp, \
             tc.tile_pool(name="btp", bufs=cfg["bt_bufs"]) as btp, \
             tc.tile_pool(name="srp", bufs=cfg["srp_bufs"]) as sp, \
             tc.tile_pool(name="redu", bufs=2) as rp, \
             tc.tile_pool(name="bwps", bufs=cfg["bw_bufs"], space="PSUM") as bwp, \
             tc.tile_pool(name="mps", bufs=cfg["mps_bufs"], space="PSUM") as mpp, \
             tc.tile_pool(name="dram", bufs=1, space="DRAM") as dp:

            # ---- one-time setup (srcw first: tile 0's gather needs it) ----
            srcw_sb = cp.tile([128, E // 16], I16)
            nc.sync.dma_start(srcw_sb[:], srcw_d.ap())
            w2_sb = cp.tile([KB, CH * 128], BF16)
            nc.sync.dma_start(w2_sb[:], w2_d.ap())
            sel4_sb = cp.tile([128, 4], BF16)
            tgtw_sb = cp.tile([128, E // 16], I16)

            atab_d = atab_din

            # edge-major messages, token-wrapped: token q at [q%128, q//128, 0:32]
            m_all = cp.tile([128, E // 128, 64], F32)
            zero_sb = cp.tile([128, (N // 128) * 64], F32)
            copies = [dp.tile([N, 64], F32, name=f"copy{c}") for c in range(NCOPY)]

            # warm the ACT activation table during idle setup
            warm_act = cp.tile([128, 4], BF16)
            nc.scalar.copy(warm_act[:], sel4_sb[:])

            def _zero_copy(c):
                nc.scalar.dma_start(
                    copies[c][:].rearrange("(p g) j -> p (g j)", p=128),
                    zero_sb[:],
                )

            def _deferred_setup():
                nc.sync.dma_start(sel4_sb[:], sel4_d.ap())
                nc.scalar.dma_start(tgtw_sb[:], tgtw_d.ap())
                # PE clock warmup: back-to-back dummy matmuls
                if cfg["warm_pe"]:
                    wps = bwp.tile([128, 2, TIL], F32, tag="bw")
                    for i in range(cfg["warm_pe"]):
                        nc.tensor.matmul(
                            out=wps[:, i % 2, :],
                            lhsT=w2_sb[:, 0:128],
                            rhs=w2_sb[:, 0:512],
                            start=True, stop=True,
                        )
                nc.gpsimd.memset(zero_sb[:], 0.0)
                # zero scatter pad cols of m_all for blocks 0-3 (tiles 0-7)
                nc.gpsimd.memset(m_all[:, 0:32, 32:64], 0.0)

            # early partial reduction state
            acc = [rp.tile([128, GQ, D], F32, tag=f"acc{q}", name=f"acc{q}")
                   for q in range(NQ)]

            def _reduce_copy(c):
                """Fold copies[c] into acc (copy c complete after tile 8c+7).

                Loads alternate between the SP and ACT HWDGE queues so the
                final copy's node-range chains run concurrently."""
                for q in range(NQ):
                    eng = (nc.sync if (q % 2 == 0 or c < NCOPY - 1)
                           else nc.scalar)
                    view = copies[c][:].rearrange("(p g) j -> p g j", p=128)[
                        :, q * GQ:(q + 1) * GQ, 0:D]
                    if c == 0:
                        eng.dma_start(acc[q][:], view)
                    else:
                        csl = rp.tile([128, GQ, D], F32, tag=f"csl{q % 2}")
                        eng.dma_start(csl[:], view)
                        add_eng = (nc.gpsimd if (c < NCOPY - 1 and
                                   cfg.get("acc_pool", True)) else nc.vector)
                        add_eng.tensor_tensor(
                            out=acc[q][:], in0=acc[q][:], in1=csl[:],
                            op=mybir.AluOpType.add,
                        )

            srep_info = {}   # tile -> (srep tile, base)
            prev = None      # (pt tiles for tile t-1, t-1)

            def _emit_gathers(t):
                for t0, ng in emit_at.get(t, []):
                    srep = sp.tile([128, 1, GATH * TIL], BF16, tag="srep")
                    nidx = ng * TIL
                    nc.gpsimd.dma_gather(
                        out_ap=srep[:, :, :nidx],
                        in_ap=atab_d.ap(),
                        idxs_ap=srcw_sb[:, t0 * (TIL // 16):(t0 + ng) * (TIL // 16)],
                        num_idxs=nidx,
                        num_idxs_reg=nidx,
                        elem_size=128,
                        transpose=True,
                    )
                    for tt in range(t0, t0 + ng):
                        srep_info[tt] = (srep, t0)

            def _emit_mproj(pts, tt):
                """Tiny-matmul projection + evac + scatter for tile tt."""
                m_ps = mpp.tile([128, 4, D], F32, tag="mps")
                for g in range(4):
                    gsl = slice(g * 128, (g + 1) * 128)
                    for c in range(CH):
                        nc.tensor.matmul(
                            out=m_ps[:, g, 4 * c:4 * c + 4],
                            lhsT=pts[c // 2][:, c % 2, gsl],
                            rhs=sel4_sb[:],
                            start=True, stop=True,
                        )
                meng = cfg["mevac"][tt % len(cfg["mevac"])]
                dst = m_all[:, tt * 4:(tt + 1) * 4, 0:D]
                if meng == "dma":
                    nc.sync.dma_start(dst, m_ps[:])
                elif meng == "act":
                    nc.scalar.copy(dst, m_ps[:])
                else:
                    nc.vector.tensor_copy(dst, m_ps[:])
                if tt >= NT - 2 and cfg.get("half_tail", True):
                    # last block: per-tile half scatters (targets within a
                    # block are unique, so the halves write disjoint rows)
                    blk = NBLK - 1
                    half = tt % 2
                    hsl = slice(blk * (TPB // 16) + half * (TPB // 32),
                                blk * (TPB // 16) + (half + 1) * (TPB // 32))
                    nc.gpsimd.dma_scatter_add(
                        out_ap=copies[NCOPY - 1][:],
                        in_ap=m_all[:, tt * 4:(tt + 1) * 4, :],
                        idxs_ap=tgtw_sb[:, hsl],
                        num_idxs=TPB // 2,
                        num_idxs_reg=TPB // 2,
                        elem_size=64,
                    )
                elif tt % 2 == 1:
                    blk = tt // 2
                    nc.gpsimd.dma_scatter_add(
                        out_ap=copies[BLK2COPY[blk]][:],
                        in_ap=m_all[:, blk * (TPB // 128):(blk + 1) * (TPB // 128), :],
                        idxs_ap=tgtw_sb[:, blk * (TPB // 16):(blk + 1) * (TPB // 16)],
                        num_idxs=TPB,
                        num_idxs_reg=TPB,
                        elem_size=64,
                    )

            # ---- main pipeline ----
            for t in range(NT):
                _emit_gathers(t)

                bt_sb = btp.tile([KB, TIL], BF16, tag="bt")
                nc.sync.dma_start(bt_sb[:], bondT_d.ap()[:, t * TIL:(t + 1) * TIL])

                if t == 0:
                    _deferred_setup()
                for c in zero_at.get(t, []):
                    _zero_copy(c)
                if t == 3:
                    nc.gpsimd.memset(m_all[:, 32:128, 32:64], 0.0)

                srep, srep_base = srep_info[t]
                ssl = slice((t - srep_base) * TIL, (t - srep_base + 1) * TIL)
                srep_half = srep[:, 0, ssl]
                srep_pair = srep[:, 0:1, ssl].to_broadcast([128, 2, TIL])

                # bw matmuls + evac/mult for tile t (PE: before mproj(t-1))
                if t < 2 and cfg.get("plan_head"):
                    plan = cfg["plan_head"]
                elif t >= NT - 2 and cfg.get("plan_tail"):
                    plan = cfg["plan_tail"]
                elif cfg.get("plans"):
                    plan = cfg["plans"][t % len(cfg["plans"])]
                else:
                    plan = cfg["plan_even"] if t % 2 == 0 else cfg["plan_odd"]
                pts = []
                for pr in range(NPAIR):
                    bw_ps = bwp.tile([128, 2, TIL], F32, tag="bw")
                    for h in range(2):
                        c = 2 * pr + h
                        nc.tensor.matmul(
                            out=bw_ps[:, h, :],
                            lhsT=w2_sb[:, c * 128:(c + 1) * 128],
                            rhs=bt_sb[:],
                            start=True, stop=True,
                        )
                    pt_sb = wp.tile([128, 2, TIL], BF16, tag="pt")
                    mode = plan[pr]
                    if mode == "D":
                        # DVE direct mult from PSUM (1x)
                        nc.vector.tensor_tensor(
                            out=pt_sb[:], in0=bw_ps[:], in1=srep_pair,
                            op=mybir.AluOpType.mult,
                        )
                    elif mode == "A":
                        # ACT pair evac + DVE pair mult (GPSIMD cannot
                        # access PSUM on HW, so evac is ACT or DVE only)
                        bw_sb = bp.tile([128, 2, TIL], BF16, tag="bwsb")
                        nc.scalar.copy(bw_sb[:], bw_ps[:])
                        nc.vector.tensor_tensor(
                            out=pt_sb[:], in0=bw_sb[:], in1=srep_pair,
                            op=mybir.AluOpType.mult,
                        )
                    elif mode == "W":
                        bw_sb = bp.tile([128, 2, TIL], BF16, tag="bwsb")
                        nc.scalar.copy(bw_sb[:], bw_ps[:])
                        nc.gpsimd.tensor_tensor(
                            out=pt_sb[:, 0, :], in0=bw_sb[:, 0, :],
                            in1=srep_half, op=mybir.AluOpType.mult,
                        )
                        nc.vector.tensor_tensor(
                            out=pt_sb[:, 1, :], in0=bw_sb[:, 1, :],
                            in1=srep_half, op=mybir.AluOpType.mult,
                        )
                    elif mode == "Y":
                        # half 0: ACT evac + DVE mult; half 1: DVE direct
                        bw_sb = bp.tile([128, 2, TIL], BF16, tag="bwsb")
                        nc.scalar.copy(bw_sb[:, 0, :], bw_ps[:, 0, :])
                        nc.vector.tensor_tensor(
                            out=pt_sb[:, 0, :], in0=bw_sb[:, 0, :],
                            in1=srep_half, op=mybir.AluOpType.mult,
                        )
                        nc.vector.tensor_tensor(
                            out=pt_sb[:, 1, :], in0=bw_ps[:, 1, :],
                            in1=srep_half, op=mybir.AluOpType.mult,
                        )
                    else:
                        raise ValueError(mode)
                    pts.append(pt_sb)
                    # mproj for tile t-1 (software-pipelined PE stream),
                    # emitted mid-tile so m_ps/m-evac land early on DVE
                    if len(pts) == cfg["mproj_pos"] and prev is not None:
                        _emit_mproj(prev[0], prev[1])
                        prev = None

                if prev is not None:
                    _emit_mproj(prev[0], prev[1])
                prev = (pts, t)

                # early copy reduction (copy c done at COPY_DONE_TILE[c])
                for c in range(NCOPY - 1):
                    if t == COPY_DONE_TILE[c] + 2:
                        _reduce_copy(c)

            # ---- drain: mproj(31), last scatter, final reduce ----
            _emit_mproj(prev[0], prev[1])
            _reduce_copy(NCOPY - 1)
            for q in range(NQ):
                nc.sync.dma_start(
                    out_d.ap().rearrange("(p g) j -> p g j", p=128)
                    [:, q * GQ:(q + 1) * GQ, :],
                    acc[q][:],
                )

    nc.compile()
    return nc


def _host_prep(atom_state, bond_state, bond_transform, connectivity):
    """Build per-core input maps. Pure layout / index-metadata / dtype prep."""
    import ml_dtypes

    W = np.asarray(bond_transform, dtype=np.float32)  # (KB, D*D)

    # W2[k, c*128 + p] = W[k, (4c + p%4)*D + p//4]   (i = 4c + p%4, j = p//4)
    p = np.arange(128)
    cc = np.arange(CH)
    i_idx = 4 * cc[:, None] + (p % 4)[None, :]   # (CH, 128)
    j_idx = np.broadcast_to((p // 4)[None, :], (CH, 128))
    w2 = W[:, (i_idx * D + j_idx).reshape(-1)].astype(ml_dtypes.bfloat16)

    # sel4[p, r] = [p % 4 == r]
    sel4 = np.zeros((128, 4), dtype=np.float32)
    sel4[p, p % 4] = 1.0
    sel4 = sel4.astype(ml_dtypes.bfloat16)

    in_maps = []
    for b in range(B):
        src = np.asarray(connectivity[b, :, 0], dtype=np.int64)
        tgt = np.asarray(connectivity[b, :, 1], dtype=np.int64)
        order = np.argsort(tgt, kind="stable")
        deg = np.bincount(tgt, minlength=N).max()
        if deg > NBLK:
            raise ValueError(f"max in-degree {deg} exceeds {NBLK}")
        # processing order: blocks by sorted_pos % NBLK
        proc = np.concatenate([order[c::NBLK] for c in range(NBLK)])
        srcp = src[proc].astype(np.int16)
        tgtp = tgt[proc].astype(np.int16)

        bondT = np.ascontiguousarray(
            np.asarray(bond_state[b], dtype=np.float32).T[:, proc]
        ).astype(ml_dtypes.bfloat16)  # (KB, E)

        # wrapped idx tables: idxs[p, s] = vals[16*s + p%16], tiled to 128 partitions
        def wrap16(vals):
            w_ = vals.reshape(-1, 16).T  # (16, E//16)
            return np.ascontiguousarray(np.tile(w_, (8, 1)), dtype=np.int16)

        # gather table T[n, 4j+r] = atom[n, j]
        atab = np.repeat(
            np.asarray(atom_state[b], dtype=np.float32), 4, axis=1
        ).astype(ml_dtypes.bfloat16)

        in_maps.append({
            "atab": np.ascontiguousarray(atab),
            "bondT": bondT,
            "w2": w2,
            "sel4": sel4,
            "srcw": wrap16(srcp),
            "tgtw": wrap16(tgtp),
        })
    return in_maps


def kernel(atom_state, bond_state, bond_transform, connectivity):
    if "nc" not in _PROGRAM_CACHE:
        _PROGRAM_CACHE["nc"] = _build_program()
    nc = _PROGRAM_CACHE["nc"]

    in_maps = _host_prep(atom_state, bond_state, bond_transform, connectivity)
    res = bass_utils.run_bass_kernel_spmd(nc, in_maps, list(range(B)))
    out = np.stack([res.results[b]["out"] for b in range(B)], axis=0)
    return out.astype(np.float32)
